# revision 10
# baseline (speedup 1.0000x reference)
"""Trainium2 Bass kernel for the 2-layer LSTM language-model problem.

Strategy (8 NeuronCores, SPMD, tensor-parallel over the 4*NN gate dim):
  - Core k owns hidden chunk k (128 of 1024 units) of both LSTM layers.
  - Transposed-gate formulation: gates are computed directly as
    [128(gate), B] PSUM tiles (weights stationary, hT streaming), so the
    new hT chunk comes out already transposed — no PE transpose — and all
    activation/elementwise ops run 128-partition wide.
  - The embedding + x-projection is folded into the recurrent matmul:
    gates += W_eff[v,m]^T @ x_t^T accumulated into the same PSUM region
    (W_eff = emb @ W0x_chunk precomputed once on-chip).
  - Per tick, TWO AllGathers (h0 then h1) with double-buffered DRAM
    bounce buffers: AG(h0(t)) is in flight while L1(t-1) computes, and
    AG(h1(t-1)) is in flight while L0(t+1) computes.
  - Gathered h1T is scattered (per-core rotated) into a DRAM history
    buffer; phase C (output MLP) runs on a 25-timestep row shard.
All matmuls run in bf16 (h crosses the wire in bf16 anyway); rel err
stays well inside the 2e-2 budget.
"""

import sys
import os

for _p in ("/opt/trn_rl_repo", "/root/.axon_site/_ro/trn_rl_repo"):
    if os.path.isdir(_p) and _p not in sys.path:
        sys.path.insert(0, _p)

import numpy as np
import ml_dtypes

BF = ml_dtypes.bfloat16

import concourse.bass as bass
import concourse.mybir as mybir
import concourse.tile as tile
from concourse import bacc
from concourse.bass_utils import run_bass_kernel_spmd

F32 = mybir.dt.float32
F32R = mybir.dt.float32r
BF16 = mybir.dt.bfloat16
I32 = mybir.dt.int32
AF = mybir.ActivationFunctionType

# problem shapes (hardcoded per contract)
T, B, V, E, NN, ON = 200, 64, 256, 512, 1024, 1024
N_CORES = 8
CH = NN // N_CORES          # 128 hidden units per core
NG = 4 * CH                 # 512 gate columns per core
KT0 = NN // 128             # 8 k-tiles for h contractions
VT = V // 128               # 2 v-tiles
ET = E // 128               # 4 e-tiles
MT = ON // 128              # 8 hid2 tiles
M_ORDER = (3, 0, 1, 2)      # gate-tile order: tanh-arg first, o last

_CACHE = {}


def _build_program(nsteps: int, variant: str = "full"):
    """Build the SPMD Bass program (identical for every core)."""
    ts_shard = nsteps // N_CORES           # phase-C timesteps per core
    rows = ts_shard * B

    nc = bacc.Bacc("TRN2", target_bir_lowering=False, debug=False,
                   num_devices=N_CORES)

    def din(name, shape, dt=BF16):
        return nc.dram_tensor(name, shape, dt, kind="ExternalInput").ap()

    xhl = din("xhl", [2 * V, nsteps * B])               # x hi/lo, replicated
    weffhl = din("weffhl", [2 * V, NG])                 # W_eff hi/lo
    w0h = din("w0h", [NN, NG])                          # lstm_w0[E:, cols_k]
    w1 = din("w1", [2 * NN, NG])                        # lstm_w1[:, cols_k]
    b0T = din("b0T", [128, 4], F32)                     # bias col per gate
    b1T = din("b1T", [128, 4], F32)
    h0T_i = din("h0T", [NN, B])                         # full h0^T, replicated
    h1T_i = din("h1T", [NN, B])
    c0T_i = din("c0T", [CH, B], F32)                    # own c chunk^T
    c1T_i = din("c1T", [CH, B], F32)
    ow0 = din("ow0", [NN, ON])                          # out_w0, replicated
    ob0 = din("ob0", [ON, 1], F32)
    ow1 = din("ow1", [ON, V])
    ob1 = din("ob1", [128, V], F32)
    scat = din("scat", [128, nsteps], I32)              # history scatter rows

    logits_out = nc.dram_tensor("logits", [rows, V], F32,
                                kind="ExternalOutput").ap()

    rg = [list(range(N_CORES))]
    use_ag = variant != "noag"

    with tile.TileContext(nc) as tc:
        with tc.tile_pool(name="dram", bufs=1, space="DRAM") as dram:
            bounceA = [dram.tile([CH, B], BF16, name=f"bounceA{i}")
                       for i in range(2)]
            bounceB = [dram.tile([CH, B], BF16, name=f"bounceB{i}")
                       for i in range(2)]
            gathA = [dram.tile([NN, B], BF16, name=f"gathA{i}")
                     for i in range(2)]
            gathB = [dram.tile([NN, B], BF16, name=f"gathB{i}")
                     for i in range(2)]
            hist_d = dram.tile([nsteps * 128, KT0 * B], BF16)

            # ------------- setup: weights resident + W_eff precompute ----
            with tc.tile_pool(name="pb_w", bufs=1) as pb_w:
                w0h_s = pb_w.tile([128, KT0 * NG], BF16)
                nc.sync.dma_start(
                    w0h_s[:].rearrange("p (k n) -> p k n", k=KT0),
                    w0h.rearrange("(k p) n -> p k n", p=128))
                w1_s = pb_w.tile([128, 2 * KT0 * NG], BF16)
                nc.sync.dma_start(
                    w1_s[:].rearrange("p (k n) -> p k n", k=2 * KT0),
                    w1.rearrange("(k p) n -> p k n", p=128))
                b0T_s = pb_w.tile([128, 4], F32)
                nc.sync.dma_start(b0T_s[:], b0T[:])
                b1T_s = pb_w.tile([128, 4], F32)
                nc.sync.dma_start(b1T_s[:], b1T[:])
                offs_s = pb_w.tile([128, nsteps], I32)
                nc.sync.dma_start(offs_s[:], scat[:])
                cT0 = pb_w.tile([CH, B], F32)
                nc.sync.dma_start(cT0[:], c0T_i[:])
                cT1 = pb_w.tile([CH, B], F32)
                nc.sync.dma_start(cT1[:], c1T_i[:])

                # W_eff = emb @ W0x_chunk (host f32, split hi/lo bf16)
                we_s = pb_w.tile([128, 2 * VT * NG], BF16)
                nc.sync.dma_start(
                    we_s[:].rearrange("p (k n) -> p k n", k=2 * VT),
                    weffhl.rearrange("(k p) n -> p k n", p=128))

                # ------------- phase B: recurrence -----------------------
                with tc.tile_pool(name="pb_g", bufs=3) as pb_g, \
                     tc.tile_pool(name="pb_x", bufs=3) as pb_x, \
                     tc.tile_pool(name="pb_ps", bufs=2, space="PSUM") as pb_ps, \
                     tc.tile_pool(name="pb_wk", bufs=2) as pb_wk:

                    G0 = pb_g.tile([128, KT0 * B], BF16, tag="G0")
                    nc.sync.dma_start(
                        G0[:].rearrange("p (k b) -> p k b", k=KT0),
                        h0T_i.rearrange("(k p) b -> p k b", p=128))
                    G1 = pb_g.tile([128, KT0 * B], BF16, tag="G1")
                    nc.sync.dma_start(
                        G1[:].rearrange("p (k b) -> p k b", k=KT0),
                        h1T_i.rearrange("(k p) b -> p k b", p=128))

                    def lstm_chain(ps, bias, cT, tag):
                        """f/i/o/g activations + c,h update; returns hT bf16."""
                        a = pb_wk.tile([128, 4 * B], F32, tag=f"a{tag}")
                        nc.scalar.activation(a[:, 3 * B:4 * B],
                                             ps[:, 3 * B:4 * B], AF.Tanh,
                                             bias=bias[:, 3:4])
                        nc.scalar.activation(a[:, 0:B], ps[:, 0:B],
                                             AF.Sigmoid, bias=bias[:, 0:1])
                        nc.scalar.activation(a[:, B:2 * B], ps[:, B:2 * B],
                                             AF.Sigmoid, bias=bias[:, 1:2])
                        nc.scalar.activation(a[:, 2 * B:3 * B],
                                             ps[:, 2 * B:3 * B],
                                             AF.Sigmoid, bias=bias[:, 2:3])
                        t1 = pb_wk.tile([128, B], F32, tag=f"t1{tag}")
                        t2 = pb_wk.tile([128, B], F32, tag=f"t2{tag}")
                        nc.vector.tensor_mul(t1[:], a[:, 0:B], cT[:])
                        nc.vector.tensor_mul(t2[:], a[:, B:2 * B],
                                             a[:, 3 * B:4 * B])
                        nc.vector.tensor_add(cT[:], t1[:], t2[:])
                        th = pb_wk.tile([128, B], F32, tag=f"th{tag}")
                        nc.scalar.activation(th[:], cT[:], AF.Tanh)
                        hT = pb_wk.tile([128, B], BF16, tag=f"h{tag}")
                        nc.vector.tensor_mul(hT[:], a[:, 2 * B:3 * B], th[:])
                        return hT

                    pend_scatter = None
                    for t in range(nsteps + 1):
                        if pend_scatter is not None:
                            g1t, ti = pend_scatter
                            nc.gpsimd.indirect_dma_start(
                                out=hist_d[:],
                                out_offset=bass.IndirectOffsetOnAxis(
                                    ap=offs_s[:, ti:ti + 1], axis=0),
                                in_=g1t[:], in_offset=None)
                            pend_scatter = None

                        do0 = t < nsteps
                        do1 = t > 0
                        G0n = G1n = None
                        if do0:
                            xt = pb_x.tile([128, 2 * VT * B], BF16, tag="xt")
                            nc.sync.dma_start(
                                xt[:].rearrange("p (k b) -> p k b", k=2 * VT),
                                xhl[:, t * B:(t + 1) * B].rearrange(
                                    "(k p) b -> p k b", p=128))
                            ps0 = pb_ps.tile([128, 4 * B], F32, tag="ps0")
                            for m in M_ORDER:
                                out = ps0[:, m * B:(m + 1) * B]
                                for kk in range(KT0):
                                    nc.tensor.matmul(
                                        out,
                                        w0h_s[:, kk * NG + m * 128:
                                              kk * NG + (m + 1) * 128],
                                        G0[:, kk * B:(kk + 1) * B],
                                        start=(kk == 0), stop=False)
                                xmms = ((0, 0), (1, 1), (2, 0), (3, 1),
                                        (0, 2), (1, 3))
                                for q, (wv, xv) in enumerate(xmms):
                                    nc.tensor.matmul(
                                        out,
                                        we_s[:, wv * NG + m * 128:
                                             wv * NG + (m + 1) * 128],
                                        xt[:, xv * B:(xv + 1) * B],
                                        start=False, stop=(q == len(xmms) - 1))
                            h0n = lstm_chain(ps0, b0T_s, cT0, "0")
                            pb = t % 2
                            nc.sync.dma_start(bounceA[pb][:], h0n[:])
                            if use_ag:
                                nc.gpsimd.collective_compute(
                                    "AllGather", mybir.AluOpType.bypass,
                                    replica_groups=rg,
                                    ins=[bounceA[pb].opt()],
                                    outs=[gathA[pb].opt()])
                            else:
                                nc.sync.dma_start(gathA[pb][0:CH, :],
                                                  bounceA[pb][:])

                        if do1:
                            ps1 = pb_ps.tile([128, 4 * B], F32, tag="ps1")
                            for m in M_ORDER:
                                out = ps1[:, m * B:(m + 1) * B]
                                for kk in range(KT0):
                                    nc.tensor.matmul(
                                        out,
                                        w1_s[:, kk * NG + m * 128:
                                             kk * NG + (m + 1) * 128],
                                        G0[:, kk * B:(kk + 1) * B],
                                        start=(kk == 0), stop=False)
                                for kk in range(KT0):
                                    nc.tensor.matmul(
                                        out,
                                        w1_s[:, (KT0 + kk) * NG + m * 128:
                                             (KT0 + kk) * NG + (m + 1) * 128],
                                        G1[:, kk * B:(kk + 1) * B],
                                        start=False, stop=(kk == KT0 - 1))
                            h1n = lstm_chain(ps1, b1T_s, cT1, "1")
                            pb = (t - 1) % 2
                            nc.sync.dma_start(bounceB[pb][:], h1n[:])
                            if use_ag:
                                nc.gpsimd.collective_compute(
                                    "AllGather", mybir.AluOpType.bypass,
                                    replica_groups=rg,
                                    ins=[bounceB[pb].opt()],
                                    outs=[gathB[pb].opt()])
                            else:
                                nc.sync.dma_start(gathB[pb][0:CH, :],
                                                  bounceB[pb][:])

                        if do0:
                            G0n = pb_g.tile([128, KT0 * B], BF16, tag="G0")
                            nc.sync.dma_start(
                                G0n[:].rearrange("p (k b) -> p k b", k=KT0),
                                gathA[t % 2].rearrange(
                                    "(k p) b -> p k b", p=128))
                        if do1:
                            G1n = pb_g.tile([128, KT0 * B], BF16, tag="G1")
                            nc.sync.dma_start(
                                G1n[:].rearrange("p (k b) -> p k b", k=KT0),
                                gathB[(t - 1) % 2].rearrange(
                                    "(k p) b -> p k b", p=128))
                            pend_scatter = (G1n, t - 1)

                        if G0n is not None:
                            G0 = G0n
                        if G1n is not None:
                            G1 = G1n

                    # final scatter (h1T(nsteps-1))
                    g1t, ti = pend_scatter
                    nc.gpsimd.indirect_dma_start(
                        out=hist_d[:],
                        out_offset=bass.IndirectOffsetOnAxis(
                            ap=offs_s[:, ti:ti + 1], axis=0),
                        in_=g1t[:], in_offset=None)

            # ---------------- phase C: output MLP on row shard ----------
            with tc.tile_pool(name="pc", bufs=2) as pc, \
                 tc.tile_pool(name="pc_ps", bufs=3, space="PSUM") as pc_ps, \
                 tc.tile_pool(name="pc_z", bufs=1) as pc_z:
                hsT = pc_z.tile([128, ts_shard * KT0 * B], BF16)
                nc.sync.dma_start(
                    hsT[:].rearrange("p (j n) -> p j n", j=ts_shard),
                    hist_d[0:ts_shard * 128, :].rearrange(
                        "(j p) n -> p j n", p=128))
                ow0_s = pc_z.tile([128, KT0 * ON], BF16)
                nc.sync.dma_start(
                    ow0_s[:].rearrange("p (k n) -> p k n", k=KT0),
                    ow0.rearrange("(k p) n -> p k n", p=128))
                ob0_s = pc_z.tile([128, MT], F32)
                nc.sync.dma_start(
                    ob0_s[:].rearrange("p (m o) -> p m o", o=1),
                    ob0.rearrange("(m p) o -> p m o", p=128))
                ow1_s = pc_z.tile([128, MT * V], BF16)
                nc.sync.dma_start(
                    ow1_s[:].rearrange("p (k n) -> p k n", k=MT),
                    ow1.rearrange("(k p) n -> p k n", p=128))
                ob1_s = pc_z.tile([128, V], F32)
                nc.sync.dma_start(ob1_s[:], ob1[:])

                z1 = pc_z.tile([128, MT * rows], BF16)           # z1T slots
                hsT_v = hsT[:].rearrange("p (j n) -> p j n", j=ts_shard)
                rgs = []
                j0 = 0
                while j0 < ts_shard:                        # 8-step groups
                    jn = min(8, ts_shard - j0)
                    rgs.append((j0, jn))
                    j0 += jn
                for m in range(MT):
                    for (j0, jn) in rgs:
                        ps = pc_ps.tile([128, 512], F32, tag="z_ps")
                        psv = ps[:, 0:jn * B].rearrange(
                            "q (j b) -> q j b", j=jn)
                        for kk in range(KT0):
                            nc.tensor.matmul(
                                psv,
                                ow0_s[:, kk * ON + m * 128:
                                      kk * ON + (m + 1) * 128],
                                hsT_v[:, j0:j0 + jn,
                                      kk * B:(kk + 1) * B],
                                start=(kk == 0), stop=(kk == KT0 - 1))
                        nc.scalar.activation(
                            z1[:, m * rows + j0 * B:
                               m * rows + (j0 + jn) * B],
                            ps[:, 0:jn * B], AF.Relu,
                            bias=ob0_s[:, m:m + 1])

                rt0 = 0
                while rt0 < rows:
                    rn = min(128, rows - rt0)
                    ps = pc_ps.tile([128, V], F32, tag="lg_ps")
                    for m in range(MT):
                        nc.tensor.matmul(
                            ps[0:rn, :],
                            z1[:, m * rows + rt0: m * rows + rt0 + rn],
                            ow1_s[:, m * V:(m + 1) * V],
                            start=(m == 0), stop=(m == MT - 1))
                    lg = pc.tile([128, V], F32, tag="lg_sb")
                    nc.vector.tensor_add(lg[0:rn, :], ps[0:rn, :],
                                         ob1_s[0:rn, :])
                    nc.sync.dma_start(logits_out[rt0:rt0 + rn, :],
                                      lg[0:rn, :])
                    rt0 += rn

    nc.compile()
    return nc


def _prep_in_maps(inputs, nsteps):
    """Slice/transpose numpy inputs into per-core input maps."""
    x = np.ascontiguousarray(inputs["inputs"], dtype=np.float32)
    x = x.reshape(nsteps * B, V)
    xT = x.T                                                  # [V, T*B] f32
    xT_hi = xT.astype(BF)
    xT_lo = (xT - xT_hi.astype(np.float32)).astype(BF)
    xhl = np.ascontiguousarray(np.concatenate([xT_hi, xT_lo], axis=0))
    emb = np.asarray(inputs["emb_matrix"], dtype=np.float32)
    w0 = np.asarray(inputs["lstm_w0"], dtype=np.float32)
    w1 = np.asarray(inputs["lstm_w1"], dtype=np.float32)
    b0 = np.asarray(inputs["lstm_b0"], dtype=np.float32)
    b1 = np.asarray(inputs["lstm_b1"], dtype=np.float32)
    h0 = np.asarray(inputs["h0"], dtype=np.float32)
    c0 = np.asarray(inputs["c0"], dtype=np.float32)
    h1 = np.asarray(inputs["h1"], dtype=np.float32)
    c1 = np.asarray(inputs["c1"], dtype=np.float32)
    h0T = np.ascontiguousarray(h0.T.astype(BF))
    h1T = np.ascontiguousarray(h1.T.astype(BF))
    ow0 = np.ascontiguousarray(
        np.asarray(inputs["out_w0"], dtype=np.float32).astype(BF))
    ob0 = np.ascontiguousarray(
        np.asarray(inputs["out_b0"], dtype=np.float32).reshape(ON, 1))
    ow1 = np.ascontiguousarray(
        np.asarray(inputs["out_w1"], dtype=np.float32).astype(BF))
    ob1 = np.ascontiguousarray(
        np.broadcast_to(inputs["out_b1"], (128, V)), dtype=np.float32)

    ts_shard = nsteps // N_CORES
    in_maps = []
    for k in range(N_CORES):
        cols = np.concatenate([
            np.arange(g * NN + k * CH, g * NN + (k + 1) * CH)
            for g in range(4)])
        rot = (np.arange(nsteps) - ts_shard * k) % nsteps
        p = np.arange(128)
        scat_rows = np.ascontiguousarray(
            (rot[None, :] * 128 + p[:, None]).astype(np.int32))
        weff = emb @ w0[:E, cols]
        weff_hi = weff.astype(BF)
        weff_lo = (weff - weff_hi.astype(np.float32)).astype(BF)
        in_maps.append({
            "xhl": xhl,
            "weffhl": np.ascontiguousarray(
                np.concatenate([weff_hi, weff_lo], axis=0)),
            "w0h": np.ascontiguousarray(w0[E:, cols].astype(BF)),
            "w1": np.ascontiguousarray(w1[:, cols].astype(BF)),
            "b0T": np.ascontiguousarray(b0[cols].reshape(4, 128).T),
            "b1T": np.ascontiguousarray(b1[cols].reshape(4, 128).T),
            "h0T": h0T,
            "h1T": h1T,
            "c0T": np.ascontiguousarray(c0[:, k * CH:(k + 1) * CH].T),
            "c1T": np.ascontiguousarray(c1[:, k * CH:(k + 1) * CH].T),
            "ow0": ow0,
            "ob0": ob0,
            "ow1": ow1,
            "ob1": ob1,
            "scat": scat_rows,
        })
    return in_maps


def kernel(**inputs):
    nsteps = inputs["inputs"].shape[0]
    if nsteps not in _CACHE:
        _CACHE[nsteps] = _build_program(nsteps)
    nc = _CACHE[nsteps]
    in_maps = _prep_in_maps(inputs, nsteps)
    res = run_bass_kernel_spmd(nc, in_maps, list(range(N_CORES)))
    logits = np.concatenate(
        [res.results[k]["logits"] for k in range(N_CORES)], axis=0)
    return logits


# revision 11
# speedup vs baseline: 1.4080x; 1.4080x over previous
"""Trainium2 Bass kernel for the 2-layer LSTM language-model problem.

Strategy (8 NeuronCores, SPMD, tensor-parallel over the 4*NN gate dim):
  - Core k owns hidden chunk k (128 of 1024 units) of both LSTM layers.
  - Transposed-gate formulation: gates are computed directly as
    [128(gate), B] PSUM tiles (weights stationary, hT streaming), so the
    new hT chunk comes out already transposed — no PE transpose — and all
    activation/elementwise ops run 128-partition wide.
  - The embedding + x-projection is folded into the recurrent matmul:
    gates += W_eff[v,m]^T @ x_t^T accumulated into the same PSUM region
    (W_eff = emb @ W0x_chunk precomputed once on-chip).
  - Per tick, TWO AllGathers (h0 then h1) with double-buffered DRAM
    bounce buffers: AG(h0(t)) is in flight while L1(t-1) computes, and
    AG(h1(t-1)) is in flight while L0(t+1) computes.
  - Gathered h1T is scattered (per-core rotated) into a DRAM history
    buffer; phase C (output MLP) runs on a 25-timestep row shard.
All matmuls run in bf16 (h crosses the wire in bf16 anyway); rel err
stays well inside the 2e-2 budget.
"""

import sys
import os

for _p in ("/opt/trn_rl_repo", "/root/.axon_site/_ro/trn_rl_repo"):
    if os.path.isdir(_p) and _p not in sys.path:
        sys.path.insert(0, _p)

import numpy as np
import ml_dtypes

BF = ml_dtypes.bfloat16

import concourse.bass as bass
import concourse.mybir as mybir
import concourse.tile as tile
from concourse import bacc
from concourse.bass_utils import run_bass_kernel_spmd

F32 = mybir.dt.float32
F32R = mybir.dt.float32r
BF16 = mybir.dt.bfloat16
I32 = mybir.dt.int32
AF = mybir.ActivationFunctionType

# problem shapes (hardcoded per contract)
T, B, V, E, NN, ON = 200, 64, 256, 512, 1024, 1024
N_CORES = 8
CH = NN // N_CORES          # 128 hidden units per core
NG = 4 * CH                 # 512 gate columns per core
KT0 = NN // 128             # 8 k-tiles for h contractions
VT = V // 128               # 2 v-tiles
ET = E // 128               # 4 e-tiles
MT = ON // 128              # 8 hid2 tiles
M_ORDER = (3, 0, 1, 2)      # gate-tile order: tanh-arg first, o last

_CACHE = {}


def _build_program(nsteps: int, variant: str = "full"):
    """Build the SPMD Bass program (identical for every core)."""
    ts_shard = nsteps // N_CORES           # phase-C timesteps per core
    rows = ts_shard * B

    nc = bacc.Bacc("TRN2", target_bir_lowering=False, debug=False,
                   num_devices=N_CORES)

    def din(name, shape, dt=BF16):
        return nc.dram_tensor(name, shape, dt, kind="ExternalInput").ap()

    xhl = din("xhl", [2 * V, nsteps * B])               # x hi/lo, replicated
    weffhl = din("weffhl", [2 * V, NG])                 # W_eff hi/lo
    w0h = din("w0h", [NN, NG])                          # lstm_w0[E:, cols_k]
    w1 = din("w1", [2 * NN, NG])                        # lstm_w1[:, cols_k]
    b0T = din("b0T", [128, 4], F32)                     # bias col per gate
    b1T = din("b1T", [128, 4], F32)
    h0T_i = din("h0T", [NN, B])                         # full h0^T, replicated
    h1T_i = din("h1T", [NN, B])
    c0T_i = din("c0T", [CH, B], F32)                    # own c chunk^T
    c1T_i = din("c1T", [CH, B], F32)
    ow0 = din("ow0", [NN, ON])                          # out_w0, replicated
    ob0 = din("ob0", [ON, 1], F32)
    ow1 = din("ow1", [ON, V])
    ob1 = din("ob1", [128, V], F32)
    scat = din("scat", [128, nsteps], I32)              # history scatter rows

    logits_out = nc.dram_tensor("logits", [rows, V], F32,
                                kind="ExternalOutput").ap()

    rg = [list(range(N_CORES))]
    use_ag = variant != "noag"

    with tile.TileContext(nc) as tc:
        with tc.tile_pool(name="dram", bufs=1, space="DRAM") as dram:
            bounceA = [dram.tile([CH, B], BF16, name=f"bounceA{i}")
                       for i in range(2)]
            bounceB = [dram.tile([CH, B], BF16, name=f"bounceB{i}")
                       for i in range(2)]
            gathA = [dram.tile([NN, B], BF16, name=f"gathA{i}")
                     for i in range(2)]
            gathB = [dram.tile([NN, B], BF16, name=f"gathB{i}")
                     for i in range(2)]
            hist_d = dram.tile([nsteps * 128, KT0 * B], BF16)

            # ------------- setup: weights resident + W_eff precompute ----
            with tc.tile_pool(name="pb_w", bufs=1) as pb_w:
                w0h_s = pb_w.tile([128, KT0 * NG], BF16)
                nc.sync.dma_start(
                    w0h_s[:].rearrange("p (k n) -> p k n", k=KT0),
                    w0h.rearrange("(k p) n -> p k n", p=128))
                w1_s = pb_w.tile([128, 2 * KT0 * NG], BF16)
                nc.sync.dma_start(
                    w1_s[:].rearrange("p (k n) -> p k n", k=2 * KT0),
                    w1.rearrange("(k p) n -> p k n", p=128))
                b0T_s = pb_w.tile([128, 4], F32)
                nc.sync.dma_start(b0T_s[:], b0T[:])
                b1T_s = pb_w.tile([128, 4], F32)
                nc.sync.dma_start(b1T_s[:], b1T[:])
                offs_s = pb_w.tile([128, nsteps], I32)
                nc.sync.dma_start(offs_s[:], scat[:])
                cT0 = pb_w.tile([CH, B], F32)
                nc.sync.dma_start(cT0[:], c0T_i[:])
                cT1 = pb_w.tile([CH, B], F32)
                nc.sync.dma_start(cT1[:], c1T_i[:])

                # W_eff = emb @ W0x_chunk (host f32, split hi/lo bf16)
                we_s = pb_w.tile([128, 2 * VT * NG], BF16)
                nc.sync.dma_start(
                    we_s[:].rearrange("p (k n) -> p k n", k=2 * VT),
                    weffhl.rearrange("(k p) n -> p k n", p=128))

                # ------------- phase B: recurrence -----------------------
                with tc.tile_pool(name="pb_g", bufs=3) as pb_g, \
                     tc.tile_pool(name="pb_x", bufs=3) as pb_x, \
                     tc.tile_pool(name="pb_ps", bufs=2, space="PSUM") as pb_ps, \
                     tc.tile_pool(name="pb_wk", bufs=2) as pb_wk:

                    G0 = pb_g.tile([128, KT0 * B], BF16, tag="G0")
                    nc.sync.dma_start(
                        G0[:].rearrange("p (k b) -> p k b", k=KT0),
                        h0T_i.rearrange("(k p) b -> p k b", p=128))
                    G1 = pb_g.tile([128, KT0 * B], BF16, tag="G1")
                    nc.sync.dma_start(
                        G1[:].rearrange("p (k b) -> p k b", k=KT0),
                        h1T_i.rearrange("(k p) b -> p k b", p=128))

                    def lstm_chain(ps, bias, cT, tag):
                        """f/i/o/g activations + c,h update; returns hT bf16."""
                        a = pb_wk.tile([128, 4 * B], F32, tag=f"a{tag}")
                        nc.scalar.activation(a[:, 3 * B:4 * B],
                                             ps[:, 3 * B:4 * B], AF.Tanh,
                                             bias=bias[:, 3:4])
                        nc.scalar.activation(a[:, 0:B], ps[:, 0:B],
                                             AF.Sigmoid, bias=bias[:, 0:1])
                        nc.scalar.activation(a[:, B:2 * B], ps[:, B:2 * B],
                                             AF.Sigmoid, bias=bias[:, 1:2])
                        nc.scalar.activation(a[:, 2 * B:3 * B],
                                             ps[:, 2 * B:3 * B],
                                             AF.Sigmoid, bias=bias[:, 2:3])
                        t1 = pb_wk.tile([128, B], F32, tag=f"t1{tag}")
                        t2 = pb_wk.tile([128, B], F32, tag=f"t2{tag}")
                        nc.vector.tensor_mul(t1[:], a[:, 0:B], cT[:])
                        nc.vector.tensor_mul(t2[:], a[:, B:2 * B],
                                             a[:, 3 * B:4 * B])
                        nc.vector.tensor_add(cT[:], t1[:], t2[:])
                        th = pb_wk.tile([128, B], F32, tag=f"th{tag}")
                        nc.scalar.activation(th[:], cT[:], AF.Tanh)
                        hT = pb_wk.tile([128, B], BF16, tag=f"h{tag}")
                        nc.vector.tensor_mul(hT[:], a[:, 2 * B:3 * B], th[:])
                        return hT

                    pend_scatter = None
                    for t in range(nsteps + 1):
                        if pend_scatter is not None:
                            g1t, ti = pend_scatter
                            nc.gpsimd.indirect_dma_start(
                                out=hist_d[:],
                                out_offset=bass.IndirectOffsetOnAxis(
                                    ap=offs_s[:, ti:ti + 1], axis=0),
                                in_=g1t[:], in_offset=None)
                            pend_scatter = None

                        do0 = t < nsteps
                        do1 = t > 0
                        G0n = G1n = None
                        if do0:
                            xt = pb_x.tile([128, 2 * VT * B], BF16, tag="xt")
                            nc.sync.dma_start(
                                xt[:].rearrange("p (k b) -> p k b", k=2 * VT),
                                xhl[:, t * B:(t + 1) * B].rearrange(
                                    "(k p) b -> p k b", p=128))
                            ps0 = pb_ps.tile([128, 4 * B], F32, tag="ps0")
                            for m in M_ORDER:
                                out = ps0[:, m * B:(m + 1) * B]
                                for kk in range(KT0):
                                    nc.tensor.matmul(
                                        out,
                                        w0h_s[:, kk * NG + m * 128:
                                              kk * NG + (m + 1) * 128],
                                        G0[:, kk * B:(kk + 1) * B],
                                        start=(kk == 0), stop=False)
                                xmms = ((0, 0), (1, 1), (2, 0), (3, 1),
                                        (0, 2), (1, 3))
                                for q, (wv, xv) in enumerate(xmms):
                                    nc.tensor.matmul(
                                        out,
                                        we_s[:, wv * NG + m * 128:
                                             wv * NG + (m + 1) * 128],
                                        xt[:, xv * B:(xv + 1) * B],
                                        start=False, stop=(q == len(xmms) - 1))
                            h0n = lstm_chain(ps0, b0T_s, cT0, "0")
                            pb = t % 2
                            nc.sync.dma_start(bounceA[pb][:], h0n[:])
                            if use_ag:
                                nc.gpsimd.collective_compute(
                                    "AllGather", mybir.AluOpType.bypass,
                                    replica_groups=rg,
                                    ins=[bounceA[pb].opt()],
                                    outs=[gathA[pb].opt()])
                            else:
                                nc.sync.dma_start(gathA[pb][0:CH, :],
                                                  bounceA[pb][:])

                        if do1:
                            ps1 = pb_ps.tile([128, 4 * B], F32, tag="ps1")
                            for m in M_ORDER:
                                out = ps1[:, m * B:(m + 1) * B]
                                for kk in range(KT0):
                                    nc.tensor.matmul(
                                        out,
                                        w1_s[:, kk * NG + m * 128:
                                             kk * NG + (m + 1) * 128],
                                        G0[:, kk * B:(kk + 1) * B],
                                        start=(kk == 0), stop=False)
                                for kk in range(KT0):
                                    nc.tensor.matmul(
                                        out,
                                        w1_s[:, (KT0 + kk) * NG + m * 128:
                                             (KT0 + kk) * NG + (m + 1) * 128],
                                        G1[:, kk * B:(kk + 1) * B],
                                        start=False, stop=(kk == KT0 - 1))
                            h1n = lstm_chain(ps1, b1T_s, cT1, "1")
                            pb = (t - 1) % 2
                            nc.sync.dma_start(bounceB[pb][:], h1n[:])
                            if use_ag:
                                nc.gpsimd.collective_compute(
                                    "AllGather", mybir.AluOpType.bypass,
                                    replica_groups=rg,
                                    ins=[bounceB[pb].opt()],
                                    outs=[gathB[pb].opt()])
                            else:
                                nc.sync.dma_start(gathB[pb][0:CH, :],
                                                  bounceB[pb][:])

                        if do0:
                            G0n = pb_g.tile([128, KT0 * B], BF16, tag="G0")
                            hk = KT0 // 2
                            nc.sync.dma_start(
                                G0n[:, 0:hk * B].rearrange(
                                    "p (k b) -> p k b", k=hk),
                                gathA[t % 2][0:hk * 128, :].rearrange(
                                    "(k p) b -> p k b", p=128))
                            nc.sync.dma_start(
                                G0n[:, hk * B:].rearrange(
                                    "p (k b) -> p k b", k=hk),
                                gathA[t % 2][hk * 128:, :].rearrange(
                                    "(k p) b -> p k b", p=128))
                        if do1:
                            G1n = pb_g.tile([128, KT0 * B], BF16, tag="G1")
                            nc.sync.dma_start(
                                G1n[:].rearrange("p (k b) -> p k b", k=KT0),
                                gathB[(t - 1) % 2].rearrange(
                                    "(k p) b -> p k b", p=128))
                            pend_scatter = (G1n, t - 1)

                        if G0n is not None:
                            G0 = G0n
                        if G1n is not None:
                            G1 = G1n

                    # final scatter (h1T(nsteps-1))
                    g1t, ti = pend_scatter
                    nc.gpsimd.indirect_dma_start(
                        out=hist_d[:],
                        out_offset=bass.IndirectOffsetOnAxis(
                            ap=offs_s[:, ti:ti + 1], axis=0),
                        in_=g1t[:], in_offset=None)

            # ---------------- phase C: output MLP on row shard ----------
            with tc.tile_pool(name="pc", bufs=2) as pc, \
                 tc.tile_pool(name="pc_ps", bufs=3, space="PSUM") as pc_ps, \
                 tc.tile_pool(name="pc_z", bufs=1) as pc_z:
                hsT = pc_z.tile([128, ts_shard * KT0 * B], BF16)
                nc.sync.dma_start(
                    hsT[:].rearrange("p (j n) -> p j n", j=ts_shard),
                    hist_d[0:ts_shard * 128, :].rearrange(
                        "(j p) n -> p j n", p=128))
                ow0_s = pc_z.tile([128, KT0 * ON], BF16)
                nc.sync.dma_start(
                    ow0_s[:].rearrange("p (k n) -> p k n", k=KT0),
                    ow0.rearrange("(k p) n -> p k n", p=128))
                ob0_s = pc_z.tile([128, MT], F32)
                nc.sync.dma_start(
                    ob0_s[:].rearrange("p (m o) -> p m o", o=1),
                    ob0.rearrange("(m p) o -> p m o", p=128))
                ow1_s = pc_z.tile([128, MT * V], BF16)
                nc.sync.dma_start(
                    ow1_s[:].rearrange("p (k n) -> p k n", k=MT),
                    ow1.rearrange("(k p) n -> p k n", p=128))
                ob1_s = pc_z.tile([128, V], F32)
                nc.sync.dma_start(ob1_s[:], ob1[:])

                z1 = pc_z.tile([128, MT * rows], BF16)           # z1T slots
                hsT_v = hsT[:].rearrange("p (j n) -> p j n", j=ts_shard)
                rgs = []
                j0 = 0
                while j0 < ts_shard:                        # 8-step groups
                    jn = min(8, ts_shard - j0)
                    rgs.append((j0, jn))
                    j0 += jn
                for m in range(MT):
                    for (j0, jn) in rgs:
                        ps = pc_ps.tile([128, 512], F32, tag="z_ps")
                        psv = ps[:, 0:jn * B].rearrange(
                            "q (j b) -> q j b", j=jn)
                        for kk in range(KT0):
                            nc.tensor.matmul(
                                psv,
                                ow0_s[:, kk * ON + m * 128:
                                      kk * ON + (m + 1) * 128],
                                hsT_v[:, j0:j0 + jn,
                                      kk * B:(kk + 1) * B],
                                start=(kk == 0), stop=(kk == KT0 - 1))
                        nc.scalar.activation(
                            z1[:, m * rows + j0 * B:
                               m * rows + (j0 + jn) * B],
                            ps[:, 0:jn * B], AF.Relu,
                            bias=ob0_s[:, m:m + 1])

                rt0 = 0
                while rt0 < rows:
                    rn = min(128, rows - rt0)
                    ps = pc_ps.tile([128, V], F32, tag="lg_ps")
                    for m in range(MT):
                        nc.tensor.matmul(
                            ps[0:rn, :],
                            z1[:, m * rows + rt0: m * rows + rt0 + rn],
                            ow1_s[:, m * V:(m + 1) * V],
                            start=(m == 0), stop=(m == MT - 1))
                    lg = pc.tile([128, V], F32, tag="lg_sb")
                    nc.vector.tensor_add(lg[0:rn, :], ps[0:rn, :],
                                         ob1_s[0:rn, :])
                    nc.sync.dma_start(logits_out[rt0:rt0 + rn, :],
                                      lg[0:rn, :])
                    rt0 += rn

    nc.compile()
    return nc


def _prep_in_maps(inputs, nsteps):
    """Slice/transpose numpy inputs into per-core input maps."""
    x = np.ascontiguousarray(inputs["inputs"], dtype=np.float32)
    x = x.reshape(nsteps * B, V)
    xT = x.T                                                  # [V, T*B] f32
    xT_hi = xT.astype(BF)
    xT_lo = (xT - xT_hi.astype(np.float32)).astype(BF)
    xhl = np.ascontiguousarray(np.concatenate([xT_hi, xT_lo], axis=0))
    emb = np.asarray(inputs["emb_matrix"], dtype=np.float32)
    w0 = np.asarray(inputs["lstm_w0"], dtype=np.float32)
    w1 = np.asarray(inputs["lstm_w1"], dtype=np.float32)
    b0 = np.asarray(inputs["lstm_b0"], dtype=np.float32)
    b1 = np.asarray(inputs["lstm_b1"], dtype=np.float32)
    h0 = np.asarray(inputs["h0"], dtype=np.float32)
    c0 = np.asarray(inputs["c0"], dtype=np.float32)
    h1 = np.asarray(inputs["h1"], dtype=np.float32)
    c1 = np.asarray(inputs["c1"], dtype=np.float32)
    h0T = np.ascontiguousarray(h0.T.astype(BF))
    h1T = np.ascontiguousarray(h1.T.astype(BF))
    ow0 = np.ascontiguousarray(
        np.asarray(inputs["out_w0"], dtype=np.float32).astype(BF))
    ob0 = np.ascontiguousarray(
        np.asarray(inputs["out_b0"], dtype=np.float32).reshape(ON, 1))
    ow1 = np.ascontiguousarray(
        np.asarray(inputs["out_w1"], dtype=np.float32).astype(BF))
    ob1 = np.ascontiguousarray(
        np.broadcast_to(inputs["out_b1"], (128, V)), dtype=np.float32)

    ts_shard = nsteps // N_CORES
    in_maps = []
    for k in range(N_CORES):
        cols = np.concatenate([
            np.arange(g * NN + k * CH, g * NN + (k + 1) * CH)
            for g in range(4)])
        rot = (np.arange(nsteps) - ts_shard * k) % nsteps
        p = np.arange(128)
        scat_rows = np.ascontiguousarray(
            (rot[None, :] * 128 + p[:, None]).astype(np.int32))
        weff = emb @ w0[:E, cols]
        weff_hi = weff.astype(BF)
        weff_lo = (weff - weff_hi.astype(np.float32)).astype(BF)
        in_maps.append({
            "xhl": xhl,
            "weffhl": np.ascontiguousarray(
                np.concatenate([weff_hi, weff_lo], axis=0)),
            "w0h": np.ascontiguousarray(w0[E:, cols].astype(BF)),
            "w1": np.ascontiguousarray(w1[:, cols].astype(BF)),
            "b0T": np.ascontiguousarray(b0[cols].reshape(4, 128).T),
            "b1T": np.ascontiguousarray(b1[cols].reshape(4, 128).T),
            "h0T": h0T,
            "h1T": h1T,
            "c0T": np.ascontiguousarray(c0[:, k * CH:(k + 1) * CH].T),
            "c1T": np.ascontiguousarray(c1[:, k * CH:(k + 1) * CH].T),
            "ow0": ow0,
            "ob0": ob0,
            "ow1": ow1,
            "ob1": ob1,
            "scat": scat_rows,
        })
    return in_maps


def kernel(**inputs):
    nsteps = inputs["inputs"].shape[0]
    if nsteps not in _CACHE:
        _CACHE[nsteps] = _build_program(nsteps)
    nc = _CACHE[nsteps]
    in_maps = _prep_in_maps(inputs, nsteps)
    res = run_bass_kernel_spmd(nc, in_maps, list(range(N_CORES)))
    logits = np.concatenate(
        [res.results[k]["logits"] for k in range(N_CORES)], axis=0)
    return logits


# revision 12
# speedup vs baseline: 1.5444x; 1.0968x over previous
"""Trainium2 Bass kernel for the 2-layer LSTM language-model problem.

Strategy (8 NeuronCores, SPMD, tensor-parallel over the 4*NN gate dim):
  - Core k owns hidden chunk k (128 of 1024 units) of both LSTM layers.
  - Transposed-gate formulation: gates are computed directly as
    [128(gate), B] PSUM tiles (weights stationary, hT streaming), so the
    new hT chunk comes out already transposed — no PE transpose — and all
    activation/elementwise ops run 128-partition wide.
  - The embedding + x-projection is folded into the recurrent matmul:
    gates += W_eff[v,m]^T @ x_t^T accumulated into the same PSUM region
    (W_eff = emb @ W0x_chunk precomputed once on-chip).
  - Per tick, TWO AllGathers (h0 then h1) with double-buffered DRAM
    bounce buffers: AG(h0(t)) is in flight while L1(t-1) computes, and
    AG(h1(t-1)) is in flight while L0(t+1) computes.
  - Gathered h1T is scattered (per-core rotated) into a DRAM history
    buffer; phase C (output MLP) runs on a 25-timestep row shard.
All matmuls run in bf16 (h crosses the wire in bf16 anyway); rel err
stays well inside the 2e-2 budget.
"""

import sys
import os

for _p in ("/opt/trn_rl_repo", "/root/.axon_site/_ro/trn_rl_repo"):
    if os.path.isdir(_p) and _p not in sys.path:
        sys.path.insert(0, _p)

import numpy as np
import ml_dtypes

BF = ml_dtypes.bfloat16

import concourse.bass as bass
import concourse.mybir as mybir
import concourse.tile as tile
from concourse import bacc
from concourse.bass_utils import run_bass_kernel_spmd

F32 = mybir.dt.float32
F32R = mybir.dt.float32r
BF16 = mybir.dt.bfloat16
I32 = mybir.dt.int32
AF = mybir.ActivationFunctionType

# problem shapes (hardcoded per contract)
T, B, V, E, NN, ON = 200, 64, 256, 512, 1024, 1024
N_CORES = 8
CH = NN // N_CORES          # 128 hidden units per core
NG = 4 * CH                 # 512 gate columns per core
KT0 = NN // 128             # 8 k-tiles for h contractions
VT = V // 128               # 2 v-tiles
ET = E // 128               # 4 e-tiles
MT = ON // 128              # 8 hid2 tiles
M_ORDER = (3, 0, 1, 2)      # gate-tile order: tanh-arg first, o last

_CACHE = {}


def _build_program(nsteps: int, variant: str = "full"):
    """Build the SPMD Bass program (identical for every core)."""
    ts_shard = nsteps // N_CORES           # phase-C timesteps per core
    rows = ts_shard * B

    nc = bacc.Bacc("TRN2", target_bir_lowering=False, debug=False,
                   num_devices=N_CORES)

    def din(name, shape, dt=BF16):
        return nc.dram_tensor(name, shape, dt, kind="ExternalInput").ap()

    xhl = din("xhl", [2 * V, nsteps * B])               # x hi/lo, replicated
    weffhl = din("weffhl", [2 * V, NG])                 # W_eff hi/lo
    w0h = din("w0h", [NN, NG])                          # lstm_w0[E:, cols_k]
    w1 = din("w1", [2 * NN, NG])                        # lstm_w1[:, cols_k]
    b0T = din("b0T", [128, 4], F32)                     # bias col per gate
    b1T = din("b1T", [128, 4], F32)
    h0T_i = din("h0T", [NN, B])                         # full h0^T, replicated
    h1T_i = din("h1T", [NN, B])
    c0T_i = din("c0T", [CH, B], F32)                    # own c chunk^T
    c1T_i = din("c1T", [CH, B], F32)
    ow0 = din("ow0", [NN, ON])                          # out_w0, replicated
    ob0 = din("ob0", [ON, 1], F32)
    ow1 = din("ow1", [ON, V])
    ob1 = din("ob1", [128, V], F32)
    scat = din("scat", [128, nsteps], I32)              # history scatter rows

    logits_out = nc.dram_tensor("logits", [rows, V], F32,
                                kind="ExternalOutput").ap()

    rg = [list(range(N_CORES))]
    use_ag = variant != "noag"

    with tile.TileContext(nc) as tc:
        with tc.tile_pool(name="dram", bufs=1, space="DRAM") as dram:
            bounceA = [dram.tile([CH, B], BF16, name=f"bounceA{i}")
                       for i in range(2)]
            bounceB = [dram.tile([CH, B], BF16, name=f"bounceB{i}")
                       for i in range(2)]
            gathA = [dram.tile([NN, B], BF16, name=f"gathA{i}")
                     for i in range(2)]
            gathB = [dram.tile([NN, B], BF16, name=f"gathB{i}")
                     for i in range(2)]
            hist_d = dram.tile([nsteps * 128, KT0 * B], BF16)

            # ------------- setup: weights resident + W_eff precompute ----
            with tc.tile_pool(name="pb_w", bufs=1) as pb_w:
                w0h_s = pb_w.tile([128, KT0 * NG], BF16)
                nc.sync.dma_start(
                    w0h_s[:].rearrange("p (k n) -> p k n", k=KT0),
                    w0h.rearrange("(k p) n -> p k n", p=128))
                w1_s = pb_w.tile([128, 2 * KT0 * NG], BF16)
                nc.sync.dma_start(
                    w1_s[:].rearrange("p (k n) -> p k n", k=2 * KT0),
                    w1.rearrange("(k p) n -> p k n", p=128))
                b0T_s = pb_w.tile([128, 4], F32)
                nc.sync.dma_start(b0T_s[:], b0T[:])
                b1T_s = pb_w.tile([128, 4], F32)
                nc.sync.dma_start(b1T_s[:], b1T[:])
                offs_s = pb_w.tile([128, nsteps], I32)
                nc.sync.dma_start(offs_s[:], scat[:])
                cT0 = pb_w.tile([CH, B], F32)
                nc.sync.dma_start(cT0[:], c0T_i[:])
                cT1 = pb_w.tile([CH, B], F32)
                nc.sync.dma_start(cT1[:], c1T_i[:])

                # W_eff = emb @ W0x_chunk (host f32, split hi/lo bf16)
                we_s = pb_w.tile([128, 2 * VT * NG], BF16)
                nc.sync.dma_start(
                    we_s[:].rearrange("p (k n) -> p k n", k=2 * VT),
                    weffhl.rearrange("(k p) n -> p k n", p=128))

                # ------------- phase B: recurrence -----------------------
                with tc.tile_pool(name="pb_g", bufs=3) as pb_g, \
                     tc.tile_pool(name="pb_x", bufs=3) as pb_x, \
                     tc.tile_pool(name="pb_ps", bufs=2, space="PSUM") as pb_ps, \
                     tc.tile_pool(name="pb_wk", bufs=2) as pb_wk:

                    G0 = pb_g.tile([128, KT0 * B], BF16, tag="G0")
                    nc.sync.dma_start(
                        G0[:].rearrange("p (k b) -> p k b", k=KT0),
                        h0T_i.rearrange("(k p) b -> p k b", p=128))
                    G1 = pb_g.tile([128, KT0 * B], BF16, tag="G1")
                    nc.sync.dma_start(
                        G1[:].rearrange("p (k b) -> p k b", k=KT0),
                        h1T_i.rearrange("(k p) b -> p k b", p=128))

                    def lstm_chain(ps, bias, cT, tag):
                        """f/i/o/g activations + c,h update; returns hT bf16."""
                        a = pb_wk.tile([128, 4 * B], F32, tag=f"a{tag}")
                        nc.scalar.activation(a[:, 3 * B:4 * B],
                                             ps[:, 3 * B:4 * B], AF.Tanh,
                                             bias=bias[:, 3:4])
                        nc.scalar.activation(a[:, 0:B], ps[:, 0:B],
                                             AF.Sigmoid, bias=bias[:, 0:1])
                        nc.scalar.activation(a[:, B:2 * B], ps[:, B:2 * B],
                                             AF.Sigmoid, bias=bias[:, 1:2])
                        nc.scalar.activation(a[:, 2 * B:3 * B],
                                             ps[:, 2 * B:3 * B],
                                             AF.Sigmoid, bias=bias[:, 2:3])
                        t1 = pb_wk.tile([128, B], F32, tag=f"t1{tag}")
                        t2 = pb_wk.tile([128, B], F32, tag=f"t2{tag}")
                        nc.vector.tensor_mul(t1[:], a[:, 0:B], cT[:])
                        nc.vector.tensor_mul(t2[:], a[:, B:2 * B],
                                             a[:, 3 * B:4 * B])
                        nc.vector.tensor_add(cT[:], t1[:], t2[:])
                        th = pb_wk.tile([128, B], F32, tag=f"th{tag}")
                        nc.scalar.activation(th[:], cT[:], AF.Tanh)
                        hT = pb_wk.tile([128, B], BF16, tag=f"h{tag}")
                        nc.vector.tensor_mul(hT[:], a[:, 2 * B:3 * B], th[:])
                        return hT

                    pend_scatter = None
                    for t in range(nsteps + 1):
                        if pend_scatter is not None:
                            g1t, ti = pend_scatter
                            nc.gpsimd.indirect_dma_start(
                                out=hist_d[:],
                                out_offset=bass.IndirectOffsetOnAxis(
                                    ap=offs_s[:, ti:ti + 1], axis=0),
                                in_=g1t[:], in_offset=None)
                            pend_scatter = None

                        do0 = t < nsteps
                        do1 = t > 0
                        G0n = G1n = None
                        if do0:
                            xt = pb_x.tile([128, 2 * VT * B], BF16, tag="xt")
                            nc.sync.dma_start(
                                xt[:].rearrange("p (k b) -> p k b", k=2 * VT),
                                xhl[:, t * B:(t + 1) * B].rearrange(
                                    "(k p) b -> p k b", p=128))
                            ps0 = pb_ps.tile([128, 4 * B], F32, tag="ps0")
                            for m in M_ORDER:
                                out = ps0[:, m * B:(m + 1) * B]
                                for kk in range(KT0):
                                    nc.tensor.matmul(
                                        out,
                                        w0h_s[:, kk * NG + m * 128:
                                              kk * NG + (m + 1) * 128],
                                        G0[:, kk * B:(kk + 1) * B],
                                        start=(kk == 0), stop=False)
                                xmms = ((0, 0), (1, 1), (2, 0), (3, 1),
                                        (0, 2), (1, 3))
                                for q, (wv, xv) in enumerate(xmms):
                                    nc.tensor.matmul(
                                        out,
                                        we_s[:, wv * NG + m * 128:
                                             wv * NG + (m + 1) * 128],
                                        xt[:, xv * B:(xv + 1) * B],
                                        start=False, stop=(q == len(xmms) - 1))
                            h0n = lstm_chain(ps0, b0T_s, cT0, "0")
                            pb = t % 2
                            nc.sync.dma_start(bounceA[pb][:], h0n[:])
                            if use_ag:
                                nc.gpsimd.collective_compute(
                                    "AllGather", mybir.AluOpType.bypass,
                                    replica_groups=rg,
                                    ins=[bounceA[pb].opt()],
                                    outs=[gathA[pb].opt()])
                            else:
                                nc.sync.dma_start(gathA[pb][0:CH, :],
                                                  bounceA[pb][:])

                        if do1:
                            ps1 = pb_ps.tile([128, 4 * B], F32, tag="ps1")
                            for m in M_ORDER:
                                out = ps1[:, m * B:(m + 1) * B]
                                for kk in range(KT0):
                                    nc.tensor.matmul(
                                        out,
                                        w1_s[:, kk * NG + m * 128:
                                             kk * NG + (m + 1) * 128],
                                        G0[:, kk * B:(kk + 1) * B],
                                        start=(kk == 0), stop=False)
                                for kk in range(KT0):
                                    nc.tensor.matmul(
                                        out,
                                        w1_s[:, (KT0 + kk) * NG + m * 128:
                                             (KT0 + kk) * NG + (m + 1) * 128],
                                        G1[:, kk * B:(kk + 1) * B],
                                        start=False, stop=(kk == KT0 - 1))
                            h1n = lstm_chain(ps1, b1T_s, cT1, "1")
                            pb = (t - 1) % 2
                            nc.sync.dma_start(bounceB[pb][:], h1n[:])
                            if use_ag:
                                nc.gpsimd.collective_compute(
                                    "AllGather", mybir.AluOpType.bypass,
                                    replica_groups=rg,
                                    ins=[bounceB[pb].opt()],
                                    outs=[gathB[pb].opt()])
                            else:
                                nc.sync.dma_start(gathB[pb][0:CH, :],
                                                  bounceB[pb][:])

                        if do0:
                            G0n = pb_g.tile([128, KT0 * B], BF16, tag="G0")
                            nc.sync.dma_start(
                                G0n[:].rearrange("p (k b) -> p k b", k=KT0),
                                gathA[t % 2].rearrange(
                                    "(k p) b -> p k b", p=128))
                        if do1:
                            G1n = pb_g.tile([128, KT0 * B], BF16, tag="G1")
                            nc.sync.dma_start(
                                G1n[:].rearrange("p (k b) -> p k b", k=KT0),
                                gathB[(t - 1) % 2].rearrange(
                                    "(k p) b -> p k b", p=128))
                            pend_scatter = (G1n, t - 1)

                        if G0n is not None:
                            G0 = G0n
                        if G1n is not None:
                            G1 = G1n

                    # final scatter (h1T(nsteps-1))
                    g1t, ti = pend_scatter
                    nc.gpsimd.indirect_dma_start(
                        out=hist_d[:],
                        out_offset=bass.IndirectOffsetOnAxis(
                            ap=offs_s[:, ti:ti + 1], axis=0),
                        in_=g1t[:], in_offset=None)

            # ---------------- phase C: output MLP on row shard ----------
            with tc.tile_pool(name="pc", bufs=2) as pc, \
                 tc.tile_pool(name="pc_ps", bufs=3, space="PSUM") as pc_ps, \
                 tc.tile_pool(name="pc_z", bufs=1) as pc_z:
                hsT = pc_z.tile([128, ts_shard * KT0 * B], BF16)
                nc.sync.dma_start(
                    hsT[:].rearrange("p (j n) -> p j n", j=ts_shard),
                    hist_d[0:ts_shard * 128, :].rearrange(
                        "(j p) n -> p j n", p=128))
                ow0_s = pc_z.tile([128, KT0 * ON], BF16)
                nc.sync.dma_start(
                    ow0_s[:].rearrange("p (k n) -> p k n", k=KT0),
                    ow0.rearrange("(k p) n -> p k n", p=128))
                ob0_s = pc_z.tile([128, MT], F32)
                nc.sync.dma_start(
                    ob0_s[:].rearrange("p (m o) -> p m o", o=1),
                    ob0.rearrange("(m p) o -> p m o", p=128))
                ow1_s = pc_z.tile([128, MT * V], BF16)
                nc.sync.dma_start(
                    ow1_s[:].rearrange("p (k n) -> p k n", k=MT),
                    ow1.rearrange("(k p) n -> p k n", p=128))
                ob1_s = pc_z.tile([128, V], F32)
                nc.sync.dma_start(ob1_s[:], ob1[:])

                z1 = pc_z.tile([128, MT * rows], BF16)           # z1T slots
                hsT_v = hsT[:].rearrange("p (j n) -> p j n", j=ts_shard)
                rgs = []
                j0 = 0
                while j0 < ts_shard:                        # 8-step groups
                    jn = min(8, ts_shard - j0)
                    rgs.append((j0, jn))
                    j0 += jn
                for m in range(MT):
                    for (j0, jn) in rgs:
                        ps = pc_ps.tile([128, 512], F32, tag="z_ps")
                        psv = ps[:, 0:jn * B].rearrange(
                            "q (j b) -> q j b", j=jn)
                        for kk in range(KT0):
                            nc.tensor.matmul(
                                psv,
                                ow0_s[:, kk * ON + m * 128:
                                      kk * ON + (m + 1) * 128],
                                hsT_v[:, j0:j0 + jn,
                                      kk * B:(kk + 1) * B],
                                start=(kk == 0), stop=(kk == KT0 - 1))
                        nc.scalar.activation(
                            z1[:, m * rows + j0 * B:
                               m * rows + (j0 + jn) * B],
                            ps[:, 0:jn * B], AF.Relu,
                            bias=ob0_s[:, m:m + 1])

                rt0 = 0
                while rt0 < rows:
                    rn = min(128, rows - rt0)
                    ps = pc_ps.tile([128, V], F32, tag="lg_ps")
                    for m in range(MT):
                        nc.tensor.matmul(
                            ps[0:rn, :],
                            z1[:, m * rows + rt0: m * rows + rt0 + rn],
                            ow1_s[:, m * V:(m + 1) * V],
                            start=(m == 0), stop=(m == MT - 1))
                    lg = pc.tile([128, V], F32, tag="lg_sb")
                    nc.vector.tensor_add(lg[0:rn, :], ps[0:rn, :],
                                         ob1_s[0:rn, :])
                    nc.sync.dma_start(logits_out[rt0:rt0 + rn, :],
                                      lg[0:rn, :])
                    rt0 += rn

    nc.compile()
    return nc


def _prep_in_maps(inputs, nsteps):
    """Slice/transpose numpy inputs into per-core input maps."""
    x = np.ascontiguousarray(inputs["inputs"], dtype=np.float32)
    x = x.reshape(nsteps * B, V)
    xT = x.T                                                  # [V, T*B] f32
    xT_hi = xT.astype(BF)
    xT_lo = (xT - xT_hi.astype(np.float32)).astype(BF)
    xhl = np.ascontiguousarray(np.concatenate([xT_hi, xT_lo], axis=0))
    emb = np.asarray(inputs["emb_matrix"], dtype=np.float32)
    w0 = np.asarray(inputs["lstm_w0"], dtype=np.float32)
    w1 = np.asarray(inputs["lstm_w1"], dtype=np.float32)
    b0 = np.asarray(inputs["lstm_b0"], dtype=np.float32)
    b1 = np.asarray(inputs["lstm_b1"], dtype=np.float32)
    h0 = np.asarray(inputs["h0"], dtype=np.float32)
    c0 = np.asarray(inputs["c0"], dtype=np.float32)
    h1 = np.asarray(inputs["h1"], dtype=np.float32)
    c1 = np.asarray(inputs["c1"], dtype=np.float32)
    h0T = np.ascontiguousarray(h0.T.astype(BF))
    h1T = np.ascontiguousarray(h1.T.astype(BF))
    ow0 = np.ascontiguousarray(
        np.asarray(inputs["out_w0"], dtype=np.float32).astype(BF))
    ob0 = np.ascontiguousarray(
        np.asarray(inputs["out_b0"], dtype=np.float32).reshape(ON, 1))
    ow1 = np.ascontiguousarray(
        np.asarray(inputs["out_w1"], dtype=np.float32).astype(BF))
    ob1 = np.ascontiguousarray(
        np.broadcast_to(inputs["out_b1"], (128, V)), dtype=np.float32)

    ts_shard = nsteps // N_CORES
    in_maps = []
    for k in range(N_CORES):
        cols = np.concatenate([
            np.arange(g * NN + k * CH, g * NN + (k + 1) * CH)
            for g in range(4)])
        rot = (np.arange(nsteps) - ts_shard * k) % nsteps
        p = np.arange(128)
        scat_rows = np.ascontiguousarray(
            (rot[None, :] * 128 + p[:, None]).astype(np.int32))
        weff = emb @ w0[:E, cols]
        weff_hi = weff.astype(BF)
        weff_lo = (weff - weff_hi.astype(np.float32)).astype(BF)
        in_maps.append({
            "xhl": xhl,
            "weffhl": np.ascontiguousarray(
                np.concatenate([weff_hi, weff_lo], axis=0)),
            "w0h": np.ascontiguousarray(w0[E:, cols].astype(BF)),
            "w1": np.ascontiguousarray(w1[:, cols].astype(BF)),
            "b0T": np.ascontiguousarray(b0[cols].reshape(4, 128).T),
            "b1T": np.ascontiguousarray(b1[cols].reshape(4, 128).T),
            "h0T": h0T,
            "h1T": h1T,
            "c0T": np.ascontiguousarray(c0[:, k * CH:(k + 1) * CH].T),
            "c1T": np.ascontiguousarray(c1[:, k * CH:(k + 1) * CH].T),
            "ow0": ow0,
            "ob0": ob0,
            "ow1": ow1,
            "ob1": ob1,
            "scat": scat_rows,
        })
    return in_maps


def kernel(**inputs):
    nsteps = inputs["inputs"].shape[0]
    if nsteps not in _CACHE:
        _CACHE[nsteps] = _build_program(nsteps)
    nc = _CACHE[nsteps]
    in_maps = _prep_in_maps(inputs, nsteps)
    res = run_bass_kernel_spmd(nc, in_maps, list(range(N_CORES)))
    logits = np.concatenate(
        [res.results[k]["logits"] for k in range(N_CORES)], axis=0)
    return logits
